# revision 5
# baseline (speedup 1.0000x reference)
"""Trainium2 Bass kernel for nn_Attention_79164837199973.

Bias-augmented multi-head self-attention with sigmoid gating.
B=4, N=1024, CQ=CH=512, H=8, D=64.

Sharding (8 cores, no collectives): core c -> batch b=c//2, query-row half
r=c%2 (512 rows). Each core computes k/v projections for the full sequence
of its batch (duplicated across the 2 cores of a batch pair -- cheaper than
an all-reduce), attention for all 8 heads over its 512 query rows, then
to_out + gating. Per-core outputs are exact disjoint shards of the result.

Math layout (all matmuls bf16, f32 PSUM accumulate):
  - scores computed transposed: sT[nk,nq] = kT_h.T-free @ qT_h via
    matmul(lhsT=kT[d,nk_chunk], rhs=qT[d,nq]); K=d=64, two heads row-packed
    into the 128x128 PE array (partitions 0-63 / 64-127).
  - softmax without max-subtraction (logits bounded ~+-7 here):
    p = exp(qk) * exp(bias), with exp(bias) precomputed on host (bf16) and
    multiplied in on VectorE (bf16 2x mode).
  - denominator via a ones-column appended to V (M=65 PV matmul, PSUM row 64).
  - per-head 1/denom broadcast across 64 partitions via a tiny PE matmul
    with a [65,64] one-hot-row matrix.
"""

import os
import sys

sys.path.insert(0, "/opt/trn_rl_repo")

import numpy as np

import concourse.bass as bass
import concourse.tile as tile
from concourse import bacc, mybir

B, N, CQ, CH, H = 4, 1024, 512, 512, 8
D = CH // H  # 64
NQ = N // 2  # 512 query rows per core
P = 128
F32 = mybir.dt.float32
BF16 = mybir.dt.bfloat16
AF = mybir.ActivationFunctionType


def build_nc():
    nc = bacc.Bacc("TRN2", target_bir_lowering=False, debug=False, num_devices=8)

    # ---- DRAM parameters (per-core shards, host-prepped) ----
    xt_e = nc.declare_dram_parameter("xt", [CQ, N], BF16, isOutput=False)
    xtq_e = nc.declare_dram_parameter("xtq", [CQ, NQ], BF16, isOutput=False)
    ebt_e = nc.declare_dram_parameter("ebt", [H, N, NQ], BF16, isOutput=False)
    wqt_e = nc.declare_dram_parameter("wqt", [CQ, CH], BF16, isOutput=False)
    wkt_e = nc.declare_dram_parameter("wkt", [CQ, CH], BF16, isOutput=False)
    wvt_e = nc.declare_dram_parameter("wvt", [CQ, CH], BF16, isOutput=False)
    wot_e = nc.declare_dram_parameter("wot", [CH, CQ], BF16, isOutput=False)
    wgt_e = nc.declare_dram_parameter("wgt", [CQ, CQ], BF16, isOutput=False)
    bqs_e = nc.declare_dram_parameter("bqs", [CH], F32, isOutput=False)
    bo_e = nc.declare_dram_parameter("bo", [CQ], F32, isOutput=False)
    gb_e = nc.declare_dram_parameter("gb", [CQ], F32, isOutput=False)
    out_e = nc.declare_dram_parameter("out", [CQ, NQ], F32, isOutput=True)

    with tile.TileContext(nc) as tc:
        with (
            tc.tile_pool(name="singles", bufs=1) as singles,
            tc.tile_pool(name="etmp", bufs=3) as etmp,
            tc.tile_pool(name="vtmp", bufs=2) as vtmp,
            tc.tile_pool(name="ps_s", bufs=2, space="PSUM") as ps_s,
            tc.tile_pool(name="ps_sm", bufs=4, space="PSUM") as ps_sm,
        ):
            # ---- persistent SBUF tiles ----
            xt_sb = singles.tile([P, 4, N], BF16)  # x^T [cq, n]
            xtq_sb = singles.tile([P, 4, NQ], BF16)  # x^T cols of our rows
            wqt_sb = singles.tile([P, 4, CH], BF16)
            wkt_sb = singles.tile([P, 4, CH], BF16)
            wvt_sb = singles.tile([P, 4, CH], BF16)
            wot_sb = singles.tile([P, 4, CQ], BF16)
            wgt_sb = singles.tile([P, 4, CQ], BF16)
            bqs_sb = singles.tile([P, 4], F32)
            bo_sb = singles.tile([P, 4], F32)
            gb_sb = singles.tile([P, 4], F32)
            ebt_sb = singles.tile([P, H * 8, NQ], BF16)  # exp(bias)^T per head
            kt_sb = singles.tile([P, 4, N], BF16)  # k^T [ch, n]
            qt_sb = singles.tile([P, 4, NQ], BF16)  # q^T [ch, nq] (scaled)
            vaug_sb = singles.tile([P, 8, H * (D + 1)], BF16)  # v rows + ones col
            ones64_sb = singles.tile([D + 1, D], BF16)  # row 64 = ones
            osc_odd = singles.tile([D, 4, NQ], BF16)  # odd heads' scaled o
            ofin_sb = singles.tile([P, 4, NQ], BF16)  # o^T [ch, nq] head-merged
            gate_sb = singles.tile([P, 4, NQ], BF16)
            outf_sb = singles.tile([P, 4, NQ], F32)

            # ---- input DMAs ----
            nc.sync.dma_start(
                out=xt_sb, in_=xt_e[:, :].rearrange("(o i) n -> i o n", i=P)
            )
            nc.sync.dma_start(
                out=xtq_sb, in_=xtq_e[:, :].rearrange("(o i) n -> i o n", i=P)
            )
            for w_sb, w_e in (
                (wkt_sb, wkt_e),
                (wqt_sb, wqt_e),
                (wvt_sb, wvt_e),
                (wot_sb, wot_e),
                (wgt_sb, wgt_e),
            ):
                nc.sync.dma_start(
                    out=w_sb, in_=w_e[:, :].rearrange("(o i) m -> i o m", i=P)
                )
            for b_sb, b_e in ((bqs_sb, bqs_e), (bo_sb, bo_e), (gb_sb, gb_e)):
                nc.sync.dma_start(
                    out=b_sb, in_=b_e[:].rearrange("(o i) -> i o", i=P)
                )
            for h in range(H):
                nc.sync.dma_start(
                    out=ebt_sb[:, h * 8 : (h + 1) * 8, :],
                    in_=ebt_e[h].rearrange("(c p) q -> p c q", p=P),
                )

            # constants
            nc.vector.memset(ones64_sb, 0.0)
            nc.vector.memset(ones64_sb[D : D + 1, :], 1.0)
            # ones column of v_aug (col D of each head's 65-wide group)
            nc.vector.memset(
                vaug_sb.rearrange("p c (h e) -> p c h e", h=H)[:, :, :, D : D + 1],
                1.0,
            )

            # ---- k^T projection: kT[ch,n] = sum_cq wkt[cq,ch] * xt[cq,n] ----
            for mo in range(4):
                for no in range(2):
                    ps = ps_sm.tile([P, 512], F32, tag="ps")
                    for ko in range(4):
                        nc.tensor.matmul(
                            ps,
                            lhsT=wkt_sb[:, ko, mo * P : (mo + 1) * P],
                            rhs=xt_sb[:, ko, no * 512 : (no + 1) * 512],
                            start=(ko == 0),
                            stop=(ko == 3),
                        )
                    nc.vector.tensor_copy(
                        out=kt_sb[:, mo, no * 512 : (no + 1) * 512], in_=ps
                    )

            # ---- q^T projection (weights pre-scaled by D^-0.5 on host) ----
            for mo in range(4):
                ps = ps_sm.tile([P, 512], F32, tag="ps")
                for ko in range(4):
                    nc.tensor.matmul(
                        ps,
                        lhsT=wqt_sb[:, ko, mo * P : (mo + 1) * P],
                        rhs=xtq_sb[:, ko, :],
                        start=(ko == 0),
                        stop=(ko == 3),
                    )
                nc.vector.tensor_scalar_add(
                    out=qt_sb[:, mo, :], in0=ps, scalar1=bqs_sb[:, mo : mo + 1]
                )

            # ---- v projection (natural layout, rows=nk) + ones column ----
            for c in range(8):
                ps = ps_sm.tile([P, 512], F32, tag="ps")
                for ko in range(4):
                    nc.tensor.matmul(
                        ps,
                        lhsT=xt_sb[:, ko, c * P : (c + 1) * P],
                        rhs=wvt_sb[:, ko, :],
                        start=(ko == 0),
                        stop=(ko == 3),
                    )
                nc.vector.tensor_copy(
                    out=vaug_sb.rearrange("p c (h e) -> p c h e", h=H)[
                        :, c, :, 0:D
                    ],
                    in_=ps.rearrange("p (h d) -> p h d", h=H),
                )

            # ---- attention, head pairs row-packed in the PE array ----
            for hp in range(4):
                heads = (2 * hp, 2 * hp + 1)
                pv_ps = {}
                for h in heads:
                    pv_ps[h] = ps_sm.tile([D + 1, NQ], F32, tag="ps", name=f"pv_{h}")
                for t in range(4):  # two nk-chunks of 128 per step
                    s_tiles = {}
                    for h in heads:
                        d0 = (h % 2) * D
                        mo = h // 2
                        s = ps_s.tile([P, 2, 512], F32, tag="s", name=f"s_{h}_{t}")
                        s_tiles[h] = s
                        for j in range(2):
                            c = 2 * t + j
                            nc.tensor.matmul(
                                s[:, j, :],
                                lhsT=kt_sb[d0 : d0 + D, mo, c * P : (c + 1) * P],
                                rhs=qt_sb[d0 : d0 + D, mo, :],
                                start=True,
                                stop=True,
                            )
                    e_tiles = {}
                    for h in heads:
                        e = etmp.tile([P, 2, 512], BF16, tag="e", name=f"e_{h}_{t}")
                        e_tiles[h] = e
                        nc.scalar.activation(out=e, in_=s_tiles[h], func=AF.Exp)
                    for h in heads:
                        nc.vector.tensor_tensor(
                            e_tiles[h],
                            e_tiles[h],
                            ebt_sb[:, h * 8 + 2 * t : h * 8 + 2 * t + 2, :],
                            mybir.AluOpType.mult,
                        )
                    for h in heads:
                        for j in range(2):
                            c = 2 * t + j
                            nc.tensor.matmul(
                                pv_ps[h],
                                lhsT=vaug_sb[:, c, h * (D + 1) : (h + 1) * (D + 1)],
                                rhs=e_tiles[h][:, j, :],
                                start=(c == 0),
                                stop=(c == 7),
                            )
                # normalize: o_h = pv[0:64] * (1/pv[64]) broadcast over 64 parts
                for h in heads:
                    recip = vtmp.tile([D + 1, NQ], BF16, tag="recip")
                    nc.vector.memset(recip[0:D, :], 0.0)
                    with nc.allow_low_precision(
                        reason="softmax denom reciprocal consumed in bf16 matmul"
                    ):
                        nc.vector.reciprocal(
                            out=recip[D : D + 1, :], in_=pv_ps[h][D : D + 1, :]
                        )
                    rbc_ps = ps_sm.tile([D, NQ], F32, tag="ps")
                    nc.tensor.matmul(
                        rbc_ps, lhsT=ones64_sb, rhs=recip, start=True, stop=True
                    )
                    rbc_sb = vtmp.tile([D, NQ], BF16, tag="rbc")
                    nc.vector.tensor_copy(out=rbc_sb, in_=rbc_ps)
                    if h % 2 == 0:
                        dst = ofin_sb[0:D, h // 2, :]
                    else:
                        dst = osc_odd[:, h // 2, :]
                    nc.vector.tensor_tensor(
                        dst, pv_ps[h][0:D, :], rbc_sb, mybir.AluOpType.mult
                    )
            # move odd heads' o into partitions 64-127 (partition relocation)
            nc.sync.dma_start(out=ofin_sb[D:P, :, :], in_=osc_odd)

            # ---- gate: sigmoid(Wg @ x^T + (bg + gating_bias)) ----
            for mo in range(4):
                ps = ps_sm.tile([P, 512], F32, tag="ps")
                for ko in range(4):
                    nc.tensor.matmul(
                        ps,
                        lhsT=wgt_sb[:, ko, mo * P : (mo + 1) * P],
                        rhs=xtq_sb[:, ko, :],
                        start=(ko == 0),
                        stop=(ko == 3),
                    )
                nc.scalar.activation(
                    out=gate_sb[:, mo, :],
                    in_=ps,
                    func=AF.Sigmoid,
                    bias=gb_sb[:, mo : mo + 1],
                )

            # ---- to_out + bo, then gate multiply ----
            for mo in range(4):
                ps = ps_sm.tile([P, 512], F32, tag="ps")
                for ko in range(4):
                    nc.tensor.matmul(
                        ps,
                        lhsT=wot_sb[:, ko, mo * P : (mo + 1) * P],
                        rhs=ofin_sb[:, ko, :],
                        start=(ko == 0),
                        stop=(ko == 3),
                    )
                tmp = etmp.tile([P, NQ], F32, tag="otmp")
                nc.vector.tensor_scalar_add(
                    out=tmp, in0=ps, scalar1=bo_sb[:, mo : mo + 1]
                )
                nc.vector.tensor_tensor(
                    outf_sb[:, mo, :], tmp, gate_sb[:, mo, :], mybir.AluOpType.mult
                )

            nc.sync.dma_start(
                out=out_e[:, :].rearrange("(o i) n -> i o n", i=P), in_=outf_sb
            )

    nc.compile()
    return nc


def make_in_maps(q_x, attn_bias, Wq, bq, Wk, Wv, Wo, bo, Wg, bg, gating_bias):
    scale = np.float32(D) ** -0.5
    import ml_dtypes

    bf16 = ml_dtypes.bfloat16

    def to_bf16(a):
        return np.ascontiguousarray(a).astype(bf16)

    wqt = to_bf16(Wq.T.astype(np.float32) * scale)
    wkt = to_bf16(Wk.T)
    wvt = to_bf16(Wv.T)
    wot = to_bf16(Wo.T)
    wgt = to_bf16(Wg.T)
    bqs = np.ascontiguousarray(bq * scale).astype(np.float32)
    bo_ = np.ascontiguousarray(bo).astype(np.float32)
    gb = np.ascontiguousarray(bg + gating_bias).astype(np.float32)

    in_maps = []
    for c in range(8):
        b, half = c // 2, c % 2
        rows = slice(half * NQ, (half + 1) * NQ)
        x = q_x[b]  # [N, CQ]
        xt = to_bf16(x.T)  # [CQ, N]
        xtq = to_bf16(x[rows].T)  # [CQ, NQ]
        ebt = to_bf16(
            np.exp(attn_bias[b, :, rows, :].astype(np.float32)).transpose(0, 2, 1)
        )  # [H, N(nk), NQ]
        in_maps.append(
            {
                "xt": xt,
                "xtq": xtq,
                "ebt": ebt,
                "wqt": wqt,
                "wkt": wkt,
                "wvt": wvt,
                "wot": wot,
                "wgt": wgt,
                "bqs": bqs,
                "bo": bo_,
                "gb": gb,
            }
        )
    return in_maps


_NC_CACHE = None


def kernel(**inputs) -> np.ndarray:
    global _NC_CACHE
    from concourse.bass_utils import run_bass_kernel_spmd

    if _NC_CACHE is None:
        _NC_CACHE = build_nc()
    nc = _NC_CACHE
    in_maps = make_in_maps(**inputs)
    trace = bool(int(os.environ.get("BASS_KERNEL_TRACE", "0")))
    res = run_bass_kernel_spmd(nc, in_maps, list(range(8)), trace=trace)
    kernel.last_result = res
    out = np.empty((B, N, CQ), dtype=np.float32)
    for c in range(8):
        b, half = c // 2, c % 2
        out[b, half * NQ : (half + 1) * NQ, :] = res.results[c]["out"].T
    return out


# revision 6
# speedup vs baseline: 1.1495x; 1.1495x over previous
"""Trainium2 Bass kernel for nn_Attention_79164837199973.

Bias-augmented multi-head self-attention with sigmoid gating.
B=4, N=1024, CQ=CH=512, H=8, D=64.

Sharding (8 cores, no collectives): core c -> batch b=c//2, query-row half
r=c%2 (512 rows). Each core computes k/v projections for the full sequence
of its batch (duplicated across the 2 cores of a batch pair -- cheaper than
an all-reduce), attention for all 8 heads over its 512 query rows, then
to_out + gating. Per-core outputs are exact disjoint shards of the result.

Layout notes (all matmuls bf16, f32 PSUM accumulate):
  - every DRAM input is host-pre-swizzled to its exact SBUF layout
    ([128 partitions, ...] with contiguous per-partition bytes) so each
    dma_start lowers to ~16 fat descriptors instead of ~1000 strided ones.
  - scores computed transposed: sT[nk,nq] via matmul(lhsT=kT[d,nk_chunk],
    rhs=qT[d,nq]); K=d=64, two heads row-packed into the PE array
    (partitions 0-63 / 64-127 concurrently).
  - softmax without max-subtraction (logits bounded ~+-7 here):
    p = exp(qk) * exp(bias), exp(bias) precomputed on host (bf16),
    multiplied in on VectorE (bf16 packed mode).
  - denominator via a ones-column appended to V (M=65 PV matmul, row 64);
    all 8 heads' reciprocals batched into ONE [8,512] DVE reciprocal
    (reciprocal costs ~6.5 cyc per per-lane element regardless of lane
    count, so 8 separate [1,512] ops would be 8x the cost).
  - per-head 1/denom broadcast over 64 partitions via tiny K=8 PE matmuls
    against a [8, 8*64] one-hot selector.
"""

import os
import sys

sys.path.insert(0, "/opt/trn_rl_repo")

import numpy as np

import concourse.bass as bass
import concourse.tile as tile
from concourse import bacc, mybir

B, N, CQ, CH, H = 4, 1024, 512, 512, 8
D = CH // H  # 64
NQ = N // 2  # 512 query rows per core
P = 128
F32 = mybir.dt.float32
BF16 = mybir.dt.bfloat16
AF = mybir.ActivationFunctionType


def build_nc():
    nc = bacc.Bacc("TRN2", target_bir_lowering=False, debug=False, num_devices=8)

    # ---- DRAM parameters, already in SBUF layout (host pre-swizzled) ----
    xt_e = nc.declare_dram_parameter("xt", [P, 4, N], BF16, isOutput=False)
    xtq_e = nc.declare_dram_parameter("xtq", [P, 4, NQ], BF16, isOutput=False)
    ebt_e = nc.declare_dram_parameter("ebt", [P, H * 8, NQ], BF16, isOutput=False)
    wqt_e = nc.declare_dram_parameter("wqt", [P, 4, CH], BF16, isOutput=False)
    wkt_e = nc.declare_dram_parameter("wkt", [P, 4, CH], BF16, isOutput=False)
    wvt_e = nc.declare_dram_parameter("wvt", [P, 4, CH], BF16, isOutput=False)
    wot_e = nc.declare_dram_parameter("wot", [P, 4, CQ], BF16, isOutput=False)
    wgt_e = nc.declare_dram_parameter("wgt", [P, 4, CQ], BF16, isOutput=False)
    bqs_e = nc.declare_dram_parameter("bqs", [P, 4], F32, isOutput=False)
    bo_e = nc.declare_dram_parameter("bo", [P, 4], F32, isOutput=False)
    gb_e = nc.declare_dram_parameter("gb", [P, 4], F32, isOutput=False)
    sel8_e = nc.declare_dram_parameter("sel8", [H, H * D], BF16, isOutput=False)
    out_e = nc.declare_dram_parameter("out", [P, 4, NQ], F32, isOutput=True)

    with tile.TileContext(nc) as tc:
        with (
            tc.tile_pool(name="singles", bufs=1) as singles,
            tc.tile_pool(name="etmp", bufs=3) as etmp,
            tc.tile_pool(name="vtmp", bufs=2) as vtmp,
            tc.tile_pool(name="ps_s", bufs=2, space="PSUM") as ps_s,
            tc.tile_pool(name="ps_sm", bufs=4, space="PSUM") as ps_sm,
        ):
            # ---- persistent SBUF tiles ----
            xt_sb = singles.tile([P, 4, N], BF16)
            xtq_sb = singles.tile([P, 4, NQ], BF16)
            wqt_sb = singles.tile([P, 4, CH], BF16)
            wkt_sb = singles.tile([P, 4, CH], BF16)
            wvt_sb = singles.tile([P, 4, CH], BF16)
            wot_sb = singles.tile([P, 4, CQ], BF16)
            wgt_sb = singles.tile([P, 4, CQ], BF16)
            bqs_sb = singles.tile([P, 4], F32)
            bo_sb = singles.tile([P, 4], F32)
            gb_sb = singles.tile([P, 4], F32)
            sel8_sb = singles.tile([H, H * D], BF16)
            ebt_sb = singles.tile([P, H * 8, NQ], BF16)
            kt_sb = singles.tile([P, 4, N], BF16)
            qt_sb = singles.tile([P, 4, NQ], BF16)
            vaug_sb = singles.tile([P, 8, H * (D + 1)], BF16)
            oraw_sb = singles.tile([D + 1, H, NQ], BF16)  # rows 0-63 o, row 64 den
            den8_sb = singles.tile([H, NQ], BF16)
            recip8_sb = singles.tile([H, NQ], BF16)
            osc_sb = singles.tile([D, H, NQ], BF16)  # normalized o, all heads
            ofin_sb = singles.tile([P, 4, NQ], BF16)  # head-merged o^T
            z_sb = singles.tile([P, 4, NQ], F32)  # gate pre-activation
            gate_sb = singles.tile([P, 4, NQ], BF16)
            outf_sb = singles.tile([P, 4, NQ], F32)

            # ---- input DMAs (all contiguous per partition) ----
            nc.sync.dma_start(out=xt_sb, in_=xt_e[:, :, :])
            nc.sync.dma_start(out=xtq_sb, in_=xtq_e[:, :, :])
            for w_sb, w_e in (
                (wkt_sb, wkt_e),
                (wqt_sb, wqt_e),
                (wvt_sb, wvt_e),
                (wot_sb, wot_e),
                (wgt_sb, wgt_e),
            ):
                nc.sync.dma_start(out=w_sb, in_=w_e[:, :, :])
            for b_sb, b_e in ((bqs_sb, bqs_e), (bo_sb, bo_e), (gb_sb, gb_e)):
                nc.sync.dma_start(out=b_sb, in_=b_e[:, :])
            nc.sync.dma_start(out=sel8_sb, in_=sel8_e[:, :])
            for h in range(H):
                nc.sync.dma_start(
                    out=ebt_sb[:, h * 8 : (h + 1) * 8, :],
                    in_=ebt_e[:, h * 8 : (h + 1) * 8, :],
                )

            # ones column of v_aug (col D of each head's 65-wide group)
            nc.vector.memset(
                vaug_sb.rearrange("p c (h e) -> p c h e", h=H)[:, :, :, D : D + 1],
                1.0,
            )

            # ---- k^T projection: kT[ch,n] = sum_cq wkt[cq,ch] * xt[cq,n] ----
            for mo in range(4):
                for no in range(2):
                    ps = ps_sm.tile([P, 512], F32, tag="ps", name="ps_k")
                    for ko in range(4):
                        nc.tensor.matmul(
                            ps,
                            lhsT=wkt_sb[:, ko, mo * P : (mo + 1) * P],
                            rhs=xt_sb[:, ko, no * 512 : (no + 1) * 512],
                            start=(ko == 0),
                            stop=(ko == 3),
                        )
                    nc.vector.tensor_copy(
                        out=kt_sb[:, mo, no * 512 : (no + 1) * 512], in_=ps
                    )

            # ---- q^T projection (weights pre-scaled by D^-0.5 on host) ----
            for mo in range(4):
                ps = ps_sm.tile([P, 512], F32, tag="ps", name="ps_q")
                for ko in range(4):
                    nc.tensor.matmul(
                        ps,
                        lhsT=wqt_sb[:, ko, mo * P : (mo + 1) * P],
                        rhs=xtq_sb[:, ko, :],
                        start=(ko == 0),
                        stop=(ko == 3),
                    )
                nc.vector.tensor_scalar_add(
                    out=qt_sb[:, mo, :], in0=ps, scalar1=bqs_sb[:, mo : mo + 1]
                )

            # ---- v projection (natural layout, rows=nk) + ones column ----
            for c in range(8):
                ps = ps_sm.tile([P, 512], F32, tag="ps", name="ps_v")
                for ko in range(4):
                    nc.tensor.matmul(
                        ps,
                        lhsT=xt_sb[:, ko, c * P : (c + 1) * P],
                        rhs=wvt_sb[:, ko, :],
                        start=(ko == 0),
                        stop=(ko == 3),
                    )
                nc.vector.tensor_copy(
                    out=vaug_sb.rearrange("p c (h e) -> p c h e", h=H)[
                        :, c, :, 0:D
                    ],
                    in_=ps.rearrange("p (h d) -> p h d", h=H),
                )

            # ---- gate pre-activation early (fills PE; sigmoid deferred) ----
            for mo in range(4):
                ps = ps_sm.tile([P, 512], F32, tag="ps", name="ps_g")
                for ko in range(4):
                    nc.tensor.matmul(
                        ps,
                        lhsT=wgt_sb[:, ko, mo * P : (mo + 1) * P],
                        rhs=xtq_sb[:, ko, :],
                        start=(ko == 0),
                        stop=(ko == 3),
                    )
                nc.vector.tensor_scalar_add(
                    out=z_sb[:, mo, :], in0=ps, scalar1=gb_sb[:, mo : mo + 1]
                )

            # ---- attention, head pairs row-packed in the PE array ----
            for hp in range(4):
                heads = (2 * hp, 2 * hp + 1)
                pv_ps = {}
                for h in heads:
                    pv_ps[h] = ps_sm.tile(
                        [D + 1, NQ], F32, tag="ps", name=f"pv_{h}"
                    )
                for t in range(4):  # two nk-chunks of 128 per step
                    s_tiles = {}
                    for h in heads:
                        d0 = (h % 2) * D
                        mo = h // 2
                        s = ps_s.tile([P, 2, 512], F32, tag="s", name=f"s_{h}_{t}")
                        s_tiles[h] = s
                        for j in range(2):
                            c = 2 * t + j
                            nc.tensor.matmul(
                                s[:, j, :],
                                lhsT=kt_sb[d0 : d0 + D, mo, c * P : (c + 1) * P],
                                rhs=qt_sb[d0 : d0 + D, mo, :],
                                start=True,
                                stop=True,
                            )
                    e_tiles = {}
                    for h in heads:
                        e = etmp.tile([P, 2, 512], BF16, tag="e", name=f"e_{h}_{t}")
                        e_tiles[h] = e
                        nc.scalar.activation(out=e, in_=s_tiles[h], func=AF.Exp)
                    for h in heads:
                        nc.vector.tensor_tensor(
                            e_tiles[h],
                            e_tiles[h],
                            ebt_sb[:, h * 8 + 2 * t : h * 8 + 2 * t + 2, :],
                            mybir.AluOpType.mult,
                        )
                    for h in heads:
                        for j in range(2):
                            c = 2 * t + j
                            nc.tensor.matmul(
                                pv_ps[h],
                                lhsT=vaug_sb[:, c, h * (D + 1) : (h + 1) * (D + 1)],
                                rhs=e_tiles[h][:, j, :],
                                start=(c == 0),
                                stop=(c == 7),
                            )
                # evacuate PSUM fast: one ACT copy grabs o rows AND den row
                for h in heads:
                    nc.scalar.copy(out=oraw_sb[:, h, :], in_=pv_ps[h])

            # ---- batched softmax denominators ----
            nc.sync.dma_start(out=den8_sb, in_=oraw_sb[D : D + 1, :, :])
            with nc.allow_low_precision(reason="softmax denom recip in bf16"):
                nc.vector.reciprocal(out=recip8_sb, in_=den8_sb)
            for h in range(H):
                rbc_ps = ps_sm.tile([D, NQ], F32, tag="ps", name=f"rbc_{h}")
                nc.tensor.matmul(
                    rbc_ps,
                    lhsT=sel8_sb[:, h * D : (h + 1) * D],
                    rhs=recip8_sb,
                    start=True,
                    stop=True,
                )
                rbc_sb = vtmp.tile([D, NQ], BF16, tag="rbc", name=f"rbc_sb_{h}")
                nc.scalar.copy(out=rbc_sb, in_=rbc_ps)
                nc.vector.tensor_tensor(
                    osc_sb[:, h, :],
                    oraw_sb[0:D, h, :],
                    rbc_sb,
                    mybir.AluOpType.mult,
                )
            # relocate odd heads to partitions 64-127 (head-merge layout)
            nc.sync.dma_start(
                out=ofin_sb[0:D, :, :],
                in_=osc_sb.rearrange("p (m t) q -> p m t q", t=2)[:, :, 0, :],
            )
            nc.sync.dma_start(
                out=ofin_sb[D:P, :, :],
                in_=osc_sb.rearrange("p (m t) q -> p m t q", t=2)[:, :, 1, :],
            )

            # ---- sigmoid gate (single big ACT op, after all exps) ----
            nc.scalar.activation(out=gate_sb, in_=z_sb, func=AF.Sigmoid)

            # ---- to_out + bo, then gate multiply ----
            for mo in range(4):
                ps = ps_sm.tile([P, 512], F32, tag="ps", name="ps_o")
                for ko in range(4):
                    nc.tensor.matmul(
                        ps,
                        lhsT=wot_sb[:, ko, mo * P : (mo + 1) * P],
                        rhs=ofin_sb[:, ko, :],
                        start=(ko == 0),
                        stop=(ko == 3),
                    )
                tmp = etmp.tile([P, NQ], F32, tag="otmp", name="otmp")
                nc.vector.tensor_scalar_add(
                    out=tmp, in0=ps, scalar1=bo_sb[:, mo : mo + 1]
                )
                nc.vector.tensor_tensor(
                    outf_sb[:, mo, :], tmp, gate_sb[:, mo, :], mybir.AluOpType.mult
                )

            nc.sync.dma_start(out=out_e[:, :, :], in_=outf_sb)

    nc.compile()
    return nc


def make_in_maps(q_x, attn_bias, Wq, bq, Wk, Wv, Wo, bo, Wg, bg, gating_bias):
    import ml_dtypes

    bf16 = ml_dtypes.bfloat16
    scale = np.float32(D) ** -0.5

    def swz(a2d):
        """[512, M] -> [128, 4, M] SBUF layout (partition-inner on dim 0)."""
        m = a2d.shape[1]
        return np.ascontiguousarray(a2d.reshape(4, P, m).transpose(1, 0, 2))

    wqt = swz(Wq.T.astype(np.float32) * scale).astype(bf16)
    wkt = swz(np.asarray(Wk.T, dtype=np.float32)).astype(bf16)
    wvt = swz(np.asarray(Wv.T, dtype=np.float32)).astype(bf16)
    wot = swz(np.asarray(Wo.T, dtype=np.float32)).astype(bf16)
    wgt = swz(np.asarray(Wg.T, dtype=np.float32)).astype(bf16)
    bqs = np.ascontiguousarray((bq * scale).reshape(4, P).T).astype(np.float32)
    bo_ = np.ascontiguousarray(np.asarray(bo).reshape(4, P).T).astype(np.float32)
    gb = np.ascontiguousarray((bg + gating_bias).reshape(4, P).T).astype(np.float32)
    sel8 = np.repeat(np.eye(H, dtype=np.float32), D, axis=1).astype(bf16)

    in_maps = []
    for c in range(8):
        b, half = c // 2, c % 2
        rows = slice(half * NQ, (half + 1) * NQ)
        x = np.asarray(q_x[b], dtype=np.float32)  # [N, CQ]
        xt = swz(x.T).astype(bf16)  # [128, 4, N]
        xtq = swz(np.ascontiguousarray(x[rows].T)).astype(bf16)
        # ebt[p, h*8+c, q] = exp(bias[b, h, rows, :]).T[c*128+p, q]
        eb = np.exp(np.asarray(attn_bias[b, :, rows, :], dtype=np.float32))
        ebt = np.ascontiguousarray(
            eb.transpose(0, 2, 1).reshape(H, 8, P, NQ).transpose(2, 0, 1, 3)
        ).reshape(P, H * 8, NQ).astype(bf16)
        in_maps.append(
            {
                "xt": xt,
                "xtq": xtq,
                "ebt": ebt,
                "wqt": wqt,
                "wkt": wkt,
                "wvt": wvt,
                "wot": wot,
                "wgt": wgt,
                "bqs": bqs,
                "bo": bo_,
                "gb": gb,
                "sel8": sel8,
            }
        )
    return in_maps


_NC_CACHE = None


def kernel(**inputs) -> np.ndarray:
    global _NC_CACHE
    from concourse.bass_utils import run_bass_kernel_spmd

    if _NC_CACHE is None:
        _NC_CACHE = build_nc()
    nc = _NC_CACHE
    in_maps = make_in_maps(**inputs)
    trace = bool(int(os.environ.get("BASS_KERNEL_TRACE", "0")))
    res = run_bass_kernel_spmd(nc, in_maps, list(range(8)), trace=trace)
    kernel.last_result = res
    out = np.empty((B, N, CQ), dtype=np.float32)
    for c in range(8):
        b, half = c // 2, c % 2
        # res "out" is [128, 4, NQ]: out^T[cq=o*128+i, q] at [i, o, q]
        o = res.results[c]["out"]
        out[b, half * NQ : (half + 1) * NQ, :] = (
            o.transpose(1, 0, 2).reshape(CQ, NQ).T
        )
    return out
